# revision 1
# baseline (speedup 1.0000x reference)
"""BlurDownsample (depthwise 4x4 FIR + 2x downsample) on 8 TRN2 NeuronCores.

Contract: kernel(x, f) takes the FULL inputs
    x: [16, 128, 256, 256] float32,  f: [4, 4] float32
and returns the FULL output [16, 128, 128, 128] float32, matching
    upfirdn2d(x, f, down=2, padding=(1, 1), flip_filter=False):
    out[n,c,oy,ox] = sum_{dy,dx in 0..3} f[3-dy, 3-dx] * xpad[2oy+dy, 2ox+dx]
with xpad zero-padded by 1 on every spatial edge.

Sharding: pure data-parallel over the batch — core k processes
x[2k:2k+2]; f is replicated. Outputs are concatenated on the host.

Per-core kernel strategy (Bass/Tile):
  - The H-direction FIR+downsample runs on the Tensor engine as banded
    matmuls in fp32r: for each filter column dx, a band matrix
        B_dx[ih, oh] = f[3-(ih-2oh+1), 3-dx]
    contracts over input rows ih (2 chunks of 128 partitions),
    producing mid_dx[oh, c, w] in PSUM at ~1 cycle/row (N = 512).
  - The W direction is a 4-way stride-2 gather-add over the mids:
    one Scalar-engine copy plus three Vector adds per channel pair,
    with the dx=0 / dx=3 edge taps clipping their ow range.
  - x is cast fp32 -> fp32r inside the load DMA (SWDGE); the band
    matrices are built on-device from f (partition-broadcast +
    affine_select band masks) so arbitrary, non-separable 4x4 filters
    are handled exactly; zero padding in H is implicit in the bands.
"""

from contextlib import ExitStack

import numpy as np

import concourse.tile as tile
from concourse import bacc, mybir
from concourse.bass_utils import run_bass_kernel_spmd

F32 = mybir.dt.float32
F32R = mybir.dt.float32r

N_CORES = 8


def _build_blur_program(nc, N, C, H, W, dt=F32R):
    OH, OW = H // 2, W // 2
    KCH = (H + 127) // 128  # contraction chunks over input rows
    PCH = min(128, H)       # partition rows per chunk
    CG = min(C, 16)         # channels per load/store group
    assert C % CG == 0 and CG % 2 == 0 and H % 2 == 0 and W % 4 == 0
    assert KCH * PCH == H and 2 * W <= 512

    x_ap = nc.dram_tensor("x", [N, C, H, W], F32, kind="ExternalInput").ap()
    f_ap = nc.dram_tensor("f", [4, 4], F32, kind="ExternalInput").ap()
    out_ap = nc.dram_tensor("out", [N, C, OH, OW], F32, kind="ExternalOutput").ap()

    with tile.TileContext(nc) as tc, ExitStack() as ctx:
        const_pool = ctx.enter_context(tc.tile_pool(name="const", bufs=1))
        x_pool = ctx.enter_context(tc.tile_pool(name="xt", bufs=3))
        acc_pool = ctx.enter_context(tc.tile_pool(name="acc", bufs=2))
        psum_pool = ctx.enter_context(tc.tile_pool(name="mid", bufs=2, space="PSUM"))

        # ---- one-time setup: broadcast f across partitions ----
        f_sb = const_pool.tile([1, 16], F32, tag="f_sb")
        nc.sync.dma_start(out=f_sb[:, :], in_=f_ap.rearrange("a b -> (a b)"))
        f_bc = const_pool.tile([128, 16], F32, tag="f_bc")
        nc.gpsimd.partition_broadcast(f_bc[:, :], f_sb[:, :])

        ones = const_pool.tile([PCH, OH], F32, tag="ones")
        nc.gpsimd.memset(ones[:, :], 1.0)

        # B[k][dx][ih_local, oh] = f[3-dy, 3-dx] where dy = ih - 2*oh + 1
        B = {}
        for k in range(KCH):
            masks = {}
            for dy in range(4):
                m = const_pool.tile([PCH, OH], F32, tag=f"mask{k}{dy}")
                nc.gpsimd.affine_select(
                    out=m[:, :],
                    in_=ones[:, :],
                    compare_op=mybir.AluOpType.is_equal,
                    fill=0.0,
                    base=128 * k + 1 - dy,
                    channel_multiplier=1,
                    pattern=[[-2, OH]],
                )
                masks[dy] = m
            for dx in range(4):
                bf = const_pool.tile([PCH, OH], F32, tag=f"Bf{k}{dx}")
                for dy in range(4):
                    fi = 4 * (3 - dy) + (3 - dx)
                    sc = f_bc[0:PCH, fi : fi + 1]
                    if dy == 0:
                        nc.vector.tensor_scalar_mul(bf[:, :], masks[0][:, :], sc)
                    else:
                        nc.vector.scalar_tensor_tensor(
                            bf[:, :],
                            masks[dy][:, :],
                            sc,
                            bf[:, :],
                            op0=mybir.AluOpType.mult,
                            op1=mybir.AluOpType.add,
                        )
                br = const_pool.tile([PCH, OH], dt, tag=f"B{k}{dx}")
                nc.gpsimd.dma_start(out=br[:, :], in_=bf[:, :])  # cast to dt
                B[(k, dx)] = br

        # ---- main loop: groups of CG channels ----
        for n in range(N):
            for c0 in range(0, C, CG):
                xt = x_pool.tile([PCH, KCH, CG, W], dt, tag="xt")
                for k in range(KCH):
                    nc.gpsimd.dma_start(  # SWDGE: casts fp32 -> dt
                        out=xt[:, k, :, :],
                        in_=x_ap[
                            n, c0 : c0 + CG, 128 * k : 128 * k + PCH
                        ].rearrange("c p w -> p c w"),
                    )
                acc = acc_pool.tile([OH, CG, OW], F32, tag="acc")
                for j in range(CG // 2):
                    mids = []
                    for dx in range(4):
                        mid = psum_pool.tile([OH, 2, W], F32, tag=f"mid{dx}")
                        for k in range(KCH):
                            nc.tensor.matmul(
                                mid[:, :, :],
                                lhsT=B[(k, dx)][:, :],
                                rhs=xt[:, k, 2 * j : 2 * j + 2, :],
                                start=(k == 0),
                                stop=(k == KCH - 1),
                            )
                        mids.append(mid)
                    a_full = acc[:, 2 * j : 2 * j + 2, :]
                    # dx=1: iw = 2ow, full range — Scalar engine (init copy)
                    nc.scalar.copy(a_full, mids[1][:, :, 0:W:2])
                    # dx=2: iw = 2ow+1, full range — Vector
                    nc.vector.tensor_add(a_full, mids[2][:, :, 1:W:2], a_full)
                    # dx=0: iw = 2ow-1, ow >= 1 — Vector
                    a0 = acc[:, 2 * j : 2 * j + 2, 1:OW]
                    nc.vector.tensor_add(a0, mids[0][:, :, 1 : W - 2 : 2], a0)
                    # dx=3: iw = 2ow+2, ow <= OW-2 — Vector
                    a3 = acc[:, 2 * j : 2 * j + 2, 0 : OW - 1]
                    nc.vector.tensor_add(a3, mids[3][:, :, 2 : W - 1 : 2], a3)
                nc.scalar.dma_start(
                    out=out_ap[n, c0 : c0 + CG].rearrange("c oh ow -> oh c ow"),
                    in_=acc[:, :, :],
                )
    return nc


_PROGRAM_CACHE = {}


def _get_program(shape):
    if shape not in _PROGRAM_CACHE:
        N, C, H, W = shape
        nb = N // N_CORES
        nc = bacc.Bacc(
            "TRN2", target_bir_lowering=False, debug=False, num_devices=N_CORES
        )
        _build_blur_program(nc, nb, C, H, W)
        nc.compile()
        _PROGRAM_CACHE[shape] = nc
    return _PROGRAM_CACHE[shape]


def _run(x, f, trace=False, tmpdir=None):
    x = np.ascontiguousarray(x, dtype=np.float32)
    f = np.ascontiguousarray(f, dtype=np.float32)
    N = x.shape[0]
    assert N % N_CORES == 0, f"batch {N} not divisible by {N_CORES} cores"
    nb = N // N_CORES
    nc = _get_program(tuple(x.shape))
    in_maps = [
        {"x": x[k * nb : (k + 1) * nb], "f": f} for k in range(N_CORES)
    ]
    res = run_bass_kernel_spmd(
        nc, in_maps, core_ids=list(range(N_CORES)), trace=trace, tmpdir=tmpdir
    )
    out = np.concatenate(
        [res.results[k]["out"] for k in range(N_CORES)], axis=0
    )
    return out, res


def kernel(x, f):
    out, _ = _run(x, f)
    return out



# revision 4
# speedup vs baseline: 1.3188x; 1.3188x over previous
"""BlurDownsample (depthwise 4x4 FIR + 2x downsample) on 8 TRN2 NeuronCores.

Contract: kernel(x, f) takes the FULL inputs
    x: [16, 128, 256, 256] float32,  f: [4, 4] float32
and returns the FULL output [16, 128, 128, 128] float32, matching
    upfirdn2d(x, f, down=2, padding=(1, 1), flip_filter=False):
    out[n,c,oy,ox] = sum_{dy,dx in 0..3} f[3-dy, 3-dx] * x[2oy+dy-1, 2ox+dx-1]
(out-of-range x indices read as zero).

Sharding: pure data-parallel over the batch - core k processes
x[2k:2k+2]; f is replicated. Outputs are reassembled on the host.

Per-core kernel strategy (Bass/Tile):
  - The WHOLE 4x4 FIR runs on the Tensor engine: for each filter tap
    column dx and each input-row parity r, a banded matrix
        B[r][dx][p, oh] = f[3-dy, 3-dx],  dy = 2p + r - 2oh + 1
    contracts input rows ih = 2p + r over the partition dim, while the
    rhs is a stride-2 slice of x columns (iw = 2ow + dx - 1), so each
    matmul emits only the needed output columns. All 8 (r, dx) matmuls
    accumulate into one PSUM tile per channel pair - no vector-engine
    W-combine at all. Edge taps (dx=0/3) write clipped ow ranges.
  - Weights are reused: the (r, dx) loop is outside the channel-pair
    sweep, so only 8 LDWEIGHTS per channel group instead of 1 per
    matmul.
  - x is cast fp32 -> bf16 in the load DMA (SWDGE) with TWO adjacent
    H rows per partition, making the HBM reads 2 KB contiguous.
  - Output is stored bf16 in [n, oh, c, ow] layout so each partition
    writes one 4 KB contiguous run; the host transposes back to
    [n, c, oh, ow] and upcasts to fp32.
"""

from contextlib import ExitStack

import numpy as np

import concourse.tile as tile
from concourse import bacc, mybir
from concourse.bass_utils import run_bass_kernel_spmd

F32 = mybir.dt.float32
BF16 = mybir.dt.bfloat16

N_CORES = 8


def _build_blur_program(nc, N, C, H, W):
    OH, OW = H // 2, W // 2
    CG = 16  # channels per load/store group
    assert C % CG == 0 and CG % 2 == 0
    assert H == 256 and W == 256, "tuned for 256x256 spatial"

    x_ap = nc.dram_tensor("x", [N, C, H, W], F32, kind="ExternalInput").ap()
    f_ap = nc.dram_tensor("f", [4, 4], F32, kind="ExternalInput").ap()
    # transposed layout: host converts [n, oh, c, ow] -> [n, c, oh, ow]
    out_ap = nc.dram_tensor("out", [N, OH, C, OW], BF16, kind="ExternalOutput").ap()

    with tile.TileContext(nc) as tc, ExitStack() as ctx:
        const_pool = ctx.enter_context(tc.tile_pool(name="const", bufs=1))
        x_pool = ctx.enter_context(tc.tile_pool(name="xt", bufs=4))
        acc_pool = ctx.enter_context(tc.tile_pool(name="acc", bufs=2))
        psum_pool = ctx.enter_context(tc.tile_pool(name="po", bufs=8, space="PSUM"))

        # ---- one-time setup: broadcast f across partitions ----
        f_sb = const_pool.tile([1, 16], F32, tag="f_sb")
        nc.sync.dma_start(out=f_sb[:, :], in_=f_ap.rearrange("a b -> (a b)"))
        f_bc = const_pool.tile([128, 16], F32, tag="f_bc")
        nc.gpsimd.partition_broadcast(f_bc[:, :], f_sb[:, :])

        ones = const_pool.tile([128, OH], F32, tag="ones")
        nc.gpsimd.memset(ones[:, :], 1.0)

        # B[r][dx][p, oh] = f[3-dy, 3-dx] where dy = 2p + r - 2oh + 1.
        # For r=0 only dy in {1,3} hit; for r=1 only dy in {0,2}.
        masks = {}
        for r in range(2):
            for dy in ((1, 3) if r == 0 else (0, 2)):
                m = const_pool.tile([128, OH], F32, tag=f"m{r}{dy}")
                nc.gpsimd.affine_select(
                    out=m[:, :],
                    in_=ones[:, :],
                    compare_op=mybir.AluOpType.is_equal,
                    fill=0.0,
                    base=r + 1 - dy,
                    channel_multiplier=2,
                    pattern=[[-2, OH]],
                )
                masks[(r, dy)] = m
        B = {}
        for r in range(2):
            dy_a, dy_b = (1, 3) if r == 0 else (0, 2)
            for dx in range(4):
                bf = const_pool.tile([128, OH], F32, tag=f"Bf{r}{dx}")
                fa = f_bc[:, 4 * (3 - dy_a) + (3 - dx) : 4 * (3 - dy_a) + (3 - dx) + 1]
                fb = f_bc[:, 4 * (3 - dy_b) + (3 - dx) : 4 * (3 - dy_b) + (3 - dx) + 1]
                nc.vector.tensor_scalar_mul(bf[:, :], masks[(r, dy_a)][:, :], fa)
                nc.vector.scalar_tensor_tensor(
                    bf[:, :],
                    masks[(r, dy_b)][:, :],
                    fb,
                    bf[:, :],
                    op0=mybir.AluOpType.mult,
                    op1=mybir.AluOpType.add,
                )
                br = const_pool.tile([128, OH], BF16, tag=f"B{r}{dx}")
                nc.gpsimd.dma_start(out=br[:, :], in_=bf[:, :])  # cast to bf16
                B[(r, dx)] = br

        # rhs w-slice start / length and psum ow-range per filter column dx
        DX_SLICE = {
            1: (0, OW, 0, OW),       # iw = 2ow,   full ow range
            2: (1, OW, 0, OW),       # iw = 2ow+1, full ow range
            0: (1, OW - 1, 1, OW),   # iw = 2ow-1, ow >= 1
            3: (2, OW - 1, 0, OW - 1),  # iw = 2ow+2, ow <= OW-2
        }
        DX_ORDER = [1, 2, 0, 3]  # first must be a full-range dx (start=True)

        # ---- main loop: groups of CG channels ----
        for n in range(N):
            for c0 in range(0, C, CG):
                # xt[p, c, r, w] holds x[n, c0+c, 2p+r, w]: 2 KB HBM runs
                xt = x_pool.tile([128, CG, 2, W], BF16, tag="xt")
                nc.gpsimd.dma_start(  # SWDGE: casts fp32 -> bf16
                    out=xt[:, :, :, :],
                    in_=x_ap[n, c0 : c0 + CG].rearrange("c (p r) w -> p c r w", r=2),
                )
                acc = acc_pool.tile([OH, CG, OW], BF16, tag="acc")
                pos = [
                    psum_pool.tile([OH, 2, OW], F32, tag="po", name=f"po{j}")
                    for j in range(CG // 2)
                ]
                for ri in range(2):
                    for di, dx in enumerate(DX_ORDER):
                        ws, wl, o0, o1 = DX_SLICE[dx]
                        lhsT = B[(ri, dx)]
                        for j in range(CG // 2):
                            nc.tensor.matmul(
                                pos[j][:, :, o0:o1],
                                lhsT=lhsT[:, :],
                                rhs=xt[
                                    :, 2 * j : 2 * j + 2, ri, ws : ws + 2 * wl - 1 : 2
                                ],
                                start=(ri == 0 and di == 0),
                                stop=(ri == 1 and di == 3),
                            )
                for j in range(CG // 2):
                    dst = acc[:, 2 * j : 2 * j + 2, :]
                    if j % 2 == 0:
                        nc.vector.tensor_copy(dst, pos[j][:, :, :])
                    else:
                        nc.scalar.copy(dst, pos[j][:, :, :])
                nc.sync.dma_start(
                    out=out_ap[n, :, c0 : c0 + CG, :], in_=acc[:, :, :]
                )
    return nc


_PROGRAM_CACHE = {}


def _get_program(shape):
    if shape not in _PROGRAM_CACHE:
        N, C, H, W = shape
        nb = N // N_CORES
        nc = bacc.Bacc(
            "TRN2", target_bir_lowering=False, debug=False, num_devices=N_CORES
        )
        _build_blur_program(nc, nb, C, H, W)
        nc.compile()
        _PROGRAM_CACHE[shape] = nc
    return _PROGRAM_CACHE[shape]


def _run(x, f, trace=False, tmpdir=None):
    x = np.ascontiguousarray(x, dtype=np.float32)
    f = np.ascontiguousarray(f, dtype=np.float32)
    N = x.shape[0]
    assert N % N_CORES == 0, f"batch {N} not divisible by {N_CORES} cores"
    nb = N // N_CORES
    nc = _get_program(tuple(x.shape))
    in_maps = [
        {"x": x[k * nb : (k + 1) * nb], "f": f} for k in range(N_CORES)
    ]
    res = run_bass_kernel_spmd(
        nc, in_maps, core_ids=list(range(N_CORES)), trace=trace, tmpdir=tmpdir
    )
    # results are [nb, OH, C, OW] bf16; reassemble to [N, C, OH, OW] fp32
    out_t = np.concatenate(
        [np.asarray(res.results[k]["out"]) for k in range(N_CORES)], axis=0
    )
    out = out_t.transpose(0, 2, 1, 3).astype(np.float32)
    return np.ascontiguousarray(out), res


def kernel(x, f):
    out, _ = _run(x, f)
    return out
